# revision 1
# baseline (speedup 1.0000x reference)
"""Trainium2 Bass kernel for nn_CLsLoss (ABCD soft-region weighted histograms +
profile likelihood).

Strategy (data-parallel over events, 8 cores):
  - Each core gets 1/8 of the 4M bkg events and 1/8 of the 4M sig events,
    reshaped to [128, COLS] (zero-padded weights for the tail).
  - Per event on-device: sigmoids s1,s2 (ScalarE, bias APs carry the runtime
    cuts), bin index idx = floor((mt-e0)/w) via ScalarE affine + the HW's
    round-to-nearest-even int16 conversion with a -0.5 bias; radix split
    q = idx>>1 (second ACT affine, -0.49 bias), s = idx - 2q.
  - Histogram via TensorE: for each 128-event column,
      psum[26,8] += Qoh[128,26]^T @ SD[128,8]
    accumulated across all 7816 columns in one PSUM fp32 group. Qoh slabs
    are tensor_scalar is_equal (23 on VectorE at 4x mode, 3 on GpSimd);
    SD1 = d (x) s via one broadcast-AP tensor_tensor, SD0 = d - SD1; weight
    channels d = (w, w*s1, w*s2, w*s1*s2) in bf16 (products on GpSimd,
    w-copy on ScalarE).
  - Host: sum per-core [26,16] partials, map (q,s)->bin, derive regions
    A=H1-H12, B=H12, C=H-H1-H2+H12, D=H2-H12, scale by INT_LUMI, and evaluate
    the tiny [50]-bin profile likelihood in float64.
"""

import numpy as np

import os as _os
SW = int(_os.environ.get("K_SW", "2"))   # s-radix width: 4 -> (13,4), 2 -> (26,2)
NBIN = 50
N_EVENTS = 4_000_000
NCORES = 8
NPC = N_EVENTS // NCORES          # 500_000 events per core per dataset
P = 128
COLS = 3908                       # 128*3908 = 500224 >= NPC (tail zero-weighted)
Q = (NBIN + SW - 1) // SW         # q = idx // SW; q>=Q naturally dropped
S = SW                            # s = idx % SW
NCH = 4                           # weight channels: 1, s1, s2, s1*s2
INT_LUMI = 117100.0
EPS = 1e-6
STEEPNESS = 20.0
CHUNK = 512


def _build_program():
    import os
    import concourse.bass as bass
    import concourse.bacc as bacc
    import concourse.mybir as mybir
    import concourse.tile as tile

    skip = set(os.environ.get("K_SKIP", "").split(","))
    lite = set(os.environ.get("K_LITE", "").split(","))
    if "idx" in skip:
        skip |= {"qoh", "soh", "sd", "mm"}
    if "sig" in skip:
        skip |= {"d", "sd", "mm"}
    if "d" in skip or "soh" in skip:
        skip |= {"sd", "mm"}
    if "sd" in skip or "qoh" in skip:
        skip |= {"mm"}
    sd_gp = int(os.environ.get("K_SD_GP", "0"))
    qoh_gp = int(os.environ.get("K_QOH_GP", "3"))
    d_eng = os.environ.get("K_D_ENG", "gpsimd")
    cp_eng = os.environ.get("K_CP_ENG", "gpsimd")
    sd_mode = os.environ.get("K_SD_MODE", "bcast4")
    soh_eng = os.environ.get("K_SOH_ENG", "vector")

    dt = mybir.dt
    Alu = mybir.AluOpType
    Act = mybir.ActivationFunctionType

    nc = bacc.Bacc("TRN2", target_bir_lowering=False, debug=False,
                   num_devices=NCORES)

    names = ["f1_bkg", "f2_bkg", "mt_bkg", "w_bkg",
             "f1_sig", "f2_sig", "mt_sig", "w_sig"]
    din = {n: nc.dram_tensor(n, [P, COLS], dt.float32, kind="ExternalInput")
           for n in names}
    dpar = nc.dram_tensor("params", [P, 8], dt.float32, kind="ExternalInput")
    dout = nc.dram_tensor("hist_out", [Q, 2 * S * NCH], dt.float32,
                          kind="ExternalOutput")

    chunks = []
    c0 = 0
    while c0 < COLS:
        chunks.append((c0, min(CHUNK, COLS - c0)))
        c0 += CHUNK

    from contextlib import ExitStack
    with tile.TileContext(nc) as tc, ExitStack() as ctx:
        io_bufs = int(os.environ.get("K_IO_BUFS", "3"))
        mid_bufs = int(os.environ.get("K_MID_BUFS", "2"))
        slab_bufs = int(os.environ.get("K_SLAB_BUFS", "2"))
        io_pool = ctx.enter_context(tc.tile_pool(name="io", bufs=io_bufs))
        mid_pool = ctx.enter_context(tc.tile_pool(name="mid", bufs=mid_bufs))
        slab_pool = ctx.enter_context(tc.tile_pool(name="slab", bufs=slab_bufs))
        const_pool = ctx.enter_context(tc.tile_pool(name="const", bufs=1))
        psum_pool = ctx.enter_context(
            tc.tile_pool(name="psum", bufs=1, space=bass.MemorySpace.PSUM))
        out_pool = ctx.enter_context(tc.tile_pool(name="out", bufs=1))

        par = const_pool.tile([P, 8], dt.float32)
        nc.sync.dma_start(par[:], dpar[:])
        bias1 = par[:, 0:1]     # -20*cut1
        bias2 = par[:, 1:2]     # -20*cut2
        invw = par[:, 2:3]      # 1/bin_width
        nege0h = par[:, 3:4]    # -edges[0]/bin_width - 0.5  (floor via rint)
        neghq = par[:, 4:5]     # -0.49 (floor of integer/4 via rint)

        psums = {}
        if "mm" not in skip:
            for ds in ("bkg", "sig"):
                psums[ds] = psum_pool.tile([Q, S * NCH], dt.float32,
                                           name=f"ps_{ds}", tag=f"ps_{ds}")

        if os.environ.get("K_ILV", "0") == "1":
            iter_order = [(ds, ci, ch) for ci, ch in enumerate(chunks)
                          for ds in ("bkg", "sig")]
        else:
            iter_order = [(ds, ci, ch) for ds in ("bkg", "sig")
                          for ci, ch in enumerate(chunks)]
        for ds, ci, (c0, tc_w) in iter_order:
            ps = psums.get(ds)
            if True:
                f1 = io_pool.tile([P, tc_w], dt.float32, tag="f1")
                f2 = io_pool.tile([P, tc_w], dt.float32, tag="f2")
                mt = io_pool.tile([P, tc_w], dt.float32, tag="mt")
                w = io_pool.tile([P, tc_w], dt.float32, tag="w")
                nc.sync.dma_start(f1[:], din[f"f1_{ds}"][:, c0:c0 + tc_w])
                nc.sync.dma_start(f2[:], din[f"f2_{ds}"][:, c0:c0 + tc_w])
                nc.sync.dma_start(mt[:], din[f"mt_{ds}"][:, c0:c0 + tc_w])
                nc.sync.dma_start(w[:], din[f"w_{ds}"][:, c0:c0 + tc_w])

                s1 = mid_pool.tile([P, tc_w], dt.bfloat16, tag="s1")
                s2 = mid_pool.tile([P, tc_w], dt.bfloat16, tag="s2")
                if "sig" not in skip:
                    nc.scalar.activation(s1[:], f1[:], Act.Sigmoid,
                                         bias=bias1, scale=STEEPNESS)
                    nc.scalar.activation(s2[:], f2[:], Act.Sigmoid,
                                         bias=bias2, scale=STEEPNESS)

                # idx = floor((mt - e0)/w) via rint(x - 0.5) on ACT -> int16
                idx16 = mid_pool.tile([P, tc_w], dt.int16, tag="idx16")
                idxbf = mid_pool.tile([P, tc_w], dt.bfloat16, tag="idxbf")
                q16 = mid_pool.tile([P, tc_w], dt.int16, tag="q16")
                _xb = int(os.environ.get("K_XBUFS", "0"))
                qbf = mid_pool.tile([P, tc_w], dt.bfloat16, tag="qbf",
                                    **({"bufs": _xb} if _xb else {}))
                q4 = mid_pool.tile([P, tc_w], dt.bfloat16, tag="q4")
                sbf = mid_pool.tile([P, tc_w], dt.bfloat16, tag="sbf",
                                    **({"bufs": _xb} if _xb else {}))
                if "idx" not in skip:
                    ceng = getattr(nc, cp_eng)
                    nc.scalar.activation(idx16[:], mt[:], Act.Identity,
                                         bias=nege0h, scale=invw)
                    ceng.tensor_copy(idxbf[:], idx16[:])
                    # q = floor(idx/4) via rint(idx*0.25 - 0.49)
                    nc.scalar.activation(q16[:], idxbf[:], Act.Identity,
                                         bias=neghq, scale=1.0 / S)
                    if os.environ.get("K_Q2", "0") == "1" and S == 2:
                        # qbf holds S*q directly; Qoh compares vs S*m
                        nc.gpsimd.tensor_scalar(qbf[:], q16[:], float(S),
                                                None, Alu.mult)
                        getattr(nc, os.environ.get("K_S_ENG", "vector")) \
                            .tensor_tensor(sbf[:], idxbf[:], qbf[:],
                                           Alu.subtract)
                    else:
                        ceng.tensor_copy(qbf[:], q16[:])
                        # s = idx - S*q  in bf16
                        nc.vector.tensor_scalar(q4[:], qbf[:], float(S), None,
                                                Alu.mult)
                        getattr(nc, os.environ.get("K_S_ENG", "vector")) \
                            .tensor_tensor(sbf[:], idxbf[:], q4[:],
                                           Alu.subtract)

                # weight channels d[.,0:4] = (w, w*s1, w*s2, w*s1*s2), bf16
                xb = int(os.environ.get("K_XBUFS", "0"))
                d = mid_pool.tile([P, NCH * tc_w], dt.bfloat16, tag="d",
                                  **({"bufs": xb} if xb else {}))
                if "d" not in skip:
                    deng = getattr(nc, d_eng)
                    if os.environ.get("K_D0", "act") == "act":
                        nc.scalar.activation(d[:, 0:tc_w], w[:], Act.Copy)
                    else:
                        nc.vector.tensor_copy(d[:, 0:tc_w], w[:])
                    deng.tensor_tensor(d[:, tc_w:2 * tc_w], d[:, 0:tc_w],
                                       s1[:], Alu.mult)
                    deng.tensor_tensor(d[:, 2 * tc_w:3 * tc_w],
                                       d[:, 0:tc_w], s2[:], Alu.mult)
                    deng.tensor_tensor(d[:, 3 * tc_w:4 * tc_w],
                                       d[:, tc_w:2 * tc_w], s2[:],
                                       Alu.mult)

                # Qoh: 13 slabs [q == m]
                qoh = slab_pool.tile([P, Q * tc_w], dt.bfloat16, tag="qoh")
                if "qoh" not in skip:
                    qmul = (float(S)
                            if os.environ.get("K_Q2", "0") == "1" and S == 2
                            else 1.0)
                    for m in range(1 if "qoh" in lite else Q):
                        eng = nc.gpsimd if m < qoh_gp else nc.vector
                        eng.tensor_scalar(
                            qoh[:, m * tc_w:(m + 1) * tc_w], qbf[:],
                            float(m) * qmul, None, Alu.is_equal)
                # Soh: 4 slabs [s == s0]
                soh = mid_pool.tile([P, S * tc_w], dt.bfloat16, tag="soh")
                if "soh" not in skip and S > 2:
                    seng = getattr(nc, soh_eng)
                    for s0 in range(1 if "soh" in lite else S):
                        seng.tensor_scalar(
                            soh[:, s0 * tc_w:(s0 + 1) * tc_w], sbf[:],
                            float(s0), None, Alu.is_equal)
                # SD: 16 slabs Soh_s0 * d_r  (slab j = s0*4 + r)
                sd = slab_pool.tile([P, S * NCH * tc_w], dt.bfloat16, tag="sd")
                if "sd" not in skip and S == 2:
                    d_b = d[:].rearrange("p (r t) -> p r t", r=NCH)
                    s_b = sbf[:].rearrange("p (o t) -> p o t", o=1)
                    s_b = s_b.to_broadcast((P, NCH, tc_w))
                    sd1 = sd[:, NCH * tc_w:2 * NCH * tc_w]
                    nc.vector.tensor_tensor(
                        sd1.rearrange("p (r t) -> p r t", r=NCH),
                        s_b, d_b, Alu.mult)
                    nc.vector.tensor_tensor(
                        sd[:, 0:NCH * tc_w], d[:, 0:NCH * tc_w],
                        sd1, Alu.subtract)
                elif "sd" not in skip:
                    if sd_mode == "bcast4":
                        d_b = d[:].rearrange("p (r t) -> p r t", r=NCH)
                        for s0 in range(S):
                            soh_b = soh[:, s0 * tc_w:(s0 + 1) * tc_w]
                            soh_b = soh_b.rearrange("p (o t) -> p o t", o=1)
                            soh_b = soh_b.to_broadcast((P, NCH, tc_w))
                            nc.vector.tensor_tensor(
                                sd[:, s0 * NCH * tc_w:(s0 + 1) * NCH * tc_w]
                                .rearrange("p (r t) -> p r t", r=NCH),
                                soh_b, d_b, Alu.mult)
                    else:
                        for s0 in range(1 if "sd" in lite else S):
                            for r in range(1 if "sd" in lite else NCH):
                                j = s0 * NCH + r
                                eng = nc.gpsimd if j < sd_gp else nc.vector
                                eng.tensor_tensor(
                                    sd[:, j * tc_w:(j + 1) * tc_w],
                                    soh[:, s0 * tc_w:(s0 + 1) * tc_w],
                                    d[:, r * tc_w:(r + 1) * tc_w], Alu.mult)

                if "mm" not in skip:
                    qoh_r = qoh[:].rearrange("p (m t) -> p t m", t=tc_w)
                    sd_r = sd[:].rearrange("p (j t) -> p t j", t=tc_w)
                    last_chunk = ci == len(chunks) - 1
                    mmstep = 8 if "mm" in lite else 1
                    for t in range(0, tc_w, mmstep):
                        nc.tensor.matmul(
                            ps[:], qoh_r[:, t, :], sd_r[:, t, :],
                            start=(ci == 0 and t == 0),
                            stop=(last_chunk and t >= tc_w - mmstep),
                            skip_group_check=True)

        out_sb = out_pool.tile([Q, 2 * S * NCH], dt.float32)
        if "mm" not in skip:
            nc.vector.tensor_copy(out_sb[:, 0:S * NCH], psums["bkg"][:])
            nc.vector.tensor_copy(out_sb[:, S * NCH:2 * S * NCH],
                                  psums["sig"][:])
            nc.sync.dma_start(dout[:], out_sb[:])

    nc.compile()
    return nc


def _shard(arr: np.ndarray, core: int) -> np.ndarray:
    sl = arr[core * NPC:(core + 1) * NPC]
    out = np.zeros(P * COLS, dtype=np.float32)
    out[:NPC] = sl
    return out.reshape(P, COLS)


def _likelihood(hb: np.ndarray, hs: np.ndarray) -> float:
    """hb/hs: [NBIN, 4] region histograms (A,B,C,D) in float64."""
    from scipy.special import gammaln

    obs_A, obs_B, obs_C, obs_D = hb[:, 0], hb[:, 1], hb[:, 2], hb[:, 3]
    S_A, S_B, S_C, S_D = hs[:, 0], hs[:, 1], hs[:, 2], hs[:, 3]
    mu = 1.0
    # theta = 0, nA/nC/nD = obs_A/obs_C/obs_D
    exp_A = obs_A + mu * S_A
    exp_C = obs_C + mu * S_C
    exp_D = obs_D + mu * S_D
    bkg_pred = obs_A * obs_D / (obs_C + EPS)
    # (1 + delta) ** theta == 1 at theta = 0
    bkg_SR = obs_A * obs_D / (obs_C + EPS)
    exp_B = bkg_SR + mu * S_B

    def pois(o, e):
        return o * np.log(e + EPS) - e - gammaln(o + 1.0)

    llh = (pois(obs_A, exp_A) + pois(obs_B, exp_B)
           + pois(obs_C, exp_C) + pois(obs_D, exp_D))
    return -float(llh.sum())


def _regions(h: np.ndarray) -> np.ndarray:
    """h: [13, 16] psum block -> [NBIN, 4] region hist (A,B,C,D) * INT_LUMI."""
    # h[q, s0*4 + r]; bin b = 4*q + s0; channels r: H, H1, H2, H12
    hq = h.reshape(Q, S, NCH)
    full = hq.reshape(Q * S, NCH)[:NBIN]  # drop bins 50,51
    H, H1, H2, H12 = full[:, 0], full[:, 1], full[:, 2], full[:, 3]
    A = H1 - H12
    B = H12
    C = H - H1 - H2 + H12
    D = H2 - H12
    return np.stack([A, B, C, D], axis=-1) * INT_LUMI


_NC_CACHE = None
LAST_RESULTS = None


def kernel(f1_bkg, f2_bkg, mt_bkg, w_bkg, f1_sig, f2_sig, mt_sig, w_sig,
           cut1, cut2, mt_bin_edges):
    global _NC_CACHE
    from concourse.bass_utils import run_bass_kernel_spmd

    if _NC_CACHE is None:
        _NC_CACHE = _build_program()
    nc = _NC_CACHE

    edges = np.asarray(mt_bin_edges, dtype=np.float64)
    width = float(edges[1] - edges[0])
    e0 = float(edges[0])
    par = np.zeros((P, 8), dtype=np.float32)
    par[:, 0] = -STEEPNESS * float(cut1)
    par[:, 1] = -STEEPNESS * float(cut2)
    par[:, 2] = 1.0 / width
    par[:, 3] = -e0 / width - 0.5
    par[:, 4] = -0.49

    arrs = {"f1_bkg": f1_bkg, "f2_bkg": f2_bkg, "mt_bkg": mt_bkg,
            "w_bkg": w_bkg, "f1_sig": f1_sig, "f2_sig": f2_sig,
            "mt_sig": mt_sig, "w_sig": w_sig}
    arrs = {k: np.asarray(v, dtype=np.float32) for k, v in arrs.items()}

    in_maps = []
    for core in range(NCORES):
        m = {k: _shard(v, core) for k, v in arrs.items()}
        m["params"] = par
        in_maps.append(m)

    try:
        res = run_bass_kernel_spmd(nc, in_maps, core_ids=list(range(NCORES)))
    except Exception:
        # transient device states (e.g. a wedged exec unit from a prior run)
        # typically clear on retry
        res = run_bass_kernel_spmd(nc, in_maps, core_ids=list(range(NCORES)))
    global LAST_RESULTS
    LAST_RESULTS = res
    total = np.zeros((Q, 2 * S * NCH), dtype=np.float64)
    for rmap in res.results:
        total += rmap["hist_out"].astype(np.float64)

    hb = _regions(total[:, 0:S * NCH])
    hs = _regions(total[:, S * NCH:2 * S * NCH])
    out = _likelihood(hb, hs)
    return np.float32(out)



# revision 5
# speedup vs baseline: 1.1280x; 1.1280x over previous
"""Trainium2 Bass kernel for nn_CLsLoss (ABCD soft-region weighted histograms +
profile likelihood).

Strategy (data-parallel over events, 8 cores):
  - Each core gets 1/8 of the 4M bkg and 1/8 of the 4M sig events as
    [128, 3908] tiles (tail zero-weighted). f1/f2/w are fed as bf16 from the
    host; mt stays fp32 (bin-edge precision).
  - Radix-2 histogram: q = idx>>1 in bf16 via the +256 rint trick
    (qf = rint(mt*invw/2 + 127.5), sf = rint(mt*invw/2 + 128) so
    s = sf - qf = idx&1, all exact in bf16's [128,256) unit-ULP window).
  - Stationary per event-column: 24 is_equal(qf, 128+m) one-hot slabs plus a
    constant ones column (bin-pair 24 is recovered on host from the ones row).
  - Moving operand: 8 channels [w, w*s1, w*s2, w*s1*s2] and the same four
    gated by s (odd-parity partial sums). Even-parity bins are recovered on
    host as M0 - M1 (no even-gated products needed on device).
  - TensorE: psum[25, 8] += qoh[128,25]^T @ dsd[128,8] per column, one PSUM
    fp32 accumulation group per dataset.
  - Host: sum per-core [25,16] partials in float64, unmix parity + ones row,
    derive regions A/B/C/D, evaluate the [50]-bin profile likelihood.
"""

import os as _os

import numpy as np

NBIN = 50
N_EVENTS = 4_000_000
NCORES = 8
NPC = N_EVENTS // NCORES          # 500_000 events per core per dataset
P = 128
COLS = 3908                       # 128*3908 = 500224 >= NPC (tail zero-weighted)
QP = 25                           # bin pairs (q = idx >> 1)
NCH = 4                           # weight channels: 1, s1, s2, s1*s2
INT_LUMI = 117100.0
EPS = 1e-6
STEEPNESS = 20.0

CHUNK = int(_os.environ.get("K_CHUNK", "1303"))
QOH_GP = int(_os.environ.get("K_QOH_GP", "2"))       # qoh slabs on GpSimd
D_GP = int(_os.environ.get("K_D_GP", "3"))           # d-products on GpSimd
SD1_ENG = _os.environ.get("K_SD1_ENG", "vector")
S_ENG = _os.environ.get("K_S_ENG", "vector")
ONES_MODE = _os.environ.get("K_ONES", "act")


def _build_program():
    import concourse.bass as bass
    import concourse.bacc as bacc
    import concourse.mybir as mybir
    import concourse.tile as tile

    dt = mybir.dt
    Alu = mybir.AluOpType
    Act = mybir.ActivationFunctionType

    nc = bacc.Bacc("TRN2", target_bir_lowering=False, debug=False,
                   num_devices=NCORES)

    din = {}
    for ds in ("bkg", "sig"):
        din[f"f1_{ds}"] = nc.dram_tensor(f"f1_{ds}", [P, COLS], dt.bfloat16,
                                         kind="ExternalInput")
        din[f"f2_{ds}"] = nc.dram_tensor(f"f2_{ds}", [P, COLS], dt.bfloat16,
                                         kind="ExternalInput")
        din[f"mt_{ds}"] = nc.dram_tensor(f"mt_{ds}", [P, COLS], dt.float32,
                                         kind="ExternalInput")
        din[f"w_{ds}"] = nc.dram_tensor(f"w_{ds}", [P, COLS], dt.bfloat16,
                                        kind="ExternalInput")
    dpar = nc.dram_tensor("params", [P, 8], dt.float32, kind="ExternalInput")
    dout = nc.dram_tensor("hist_out", [QP, 4 * NCH], dt.float32,
                          kind="ExternalOutput")

    chunks = []
    c0 = 0
    while c0 < COLS:
        chunks.append((c0, min(CHUNK, COLS - c0)))
        c0 += CHUNK

    from contextlib import ExitStack
    with tile.TileContext(nc) as tc, ExitStack() as ctx:
        io_pool = ctx.enter_context(tc.tile_pool(name="io", bufs=2))
        mid_pool = ctx.enter_context(tc.tile_pool(name="mid", bufs=2))
        qoh_pool = ctx.enter_context(tc.tile_pool(name="qoh", bufs=2))
        const_pool = ctx.enter_context(tc.tile_pool(name="const", bufs=1))
        psum_pool = ctx.enter_context(
            tc.tile_pool(name="psum", bufs=1, space=bass.MemorySpace.PSUM))
        out_pool = ctx.enter_context(tc.tile_pool(name="out", bufs=1))

        par = const_pool.tile([P, 8], dt.float32)
        nc.sync.dma_start(par[:], dpar[:])
        bias1 = par[:, 0:1]      # -20*cut1
        bias2 = par[:, 1:2]      # -20*cut2
        hinvw = par[:, 2:3]      # 0.5/bin_width
        qbias = par[:, 3:4]      # 127.5 - e0/(2w)
        sbias = par[:, 4:5]      # 128.0 - e0/(2w)

        psums = {ds: psum_pool.tile([QP, 2 * NCH], dt.float32,
                                    name=f"ps_{ds}", tag=f"ps_{ds}")
                 for ds in ("bkg", "sig")}

        for ds in ("bkg", "sig"):
            ps = psums[ds]
            for ci, (c0, cw) in enumerate(chunks):
                f1 = io_pool.tile([P, cw], dt.bfloat16, tag="f1", bufs=1)
                f2 = io_pool.tile([P, cw], dt.bfloat16, tag="f2")
                mt = io_pool.tile([P, cw], dt.float32, tag="mt")
                # dsd channels: w, w*s1, w*s2, w*s1*s2, then the same gated
                # by the parity bit s
                dsd = mid_pool.tile([P, 8 * cw], dt.bfloat16, tag="dsd")
                nc.sync.dma_start(f1[:], din[f"f1_{ds}"][:, c0:c0 + cw])
                nc.sync.dma_start(f2[:], din[f"f2_{ds}"][:, c0:c0 + cw])
                nc.sync.dma_start(mt[:], din[f"mt_{ds}"][:, c0:c0 + cw])
                nc.sync.dma_start(dsd[:, 0:cw], din[f"w_{ds}"][:, c0:c0 + cw])

                s1 = mid_pool.tile([P, cw], dt.bfloat16, tag="s1")
                s2 = mid_pool.tile([P, cw], dt.bfloat16, tag="s2")
                qf = mid_pool.tile([P, cw], dt.bfloat16, tag="qf")
                sf = mid_pool.tile([P, cw], dt.bfloat16, tag="sf", bufs=1)
                sb = mid_pool.tile([P, cw], dt.bfloat16, tag="sb", bufs=1)
                nc.scalar.activation(s1[:], f1[:], Act.Sigmoid,
                                     bias=bias1, scale=STEEPNESS)
                nc.scalar.activation(s2[:], f2[:], Act.Sigmoid,
                                     bias=bias2, scale=STEEPNESS)
                nc.scalar.activation(qf[:], mt[:], Act.Identity,
                                     bias=qbias, scale=hinvw)
                nc.scalar.activation(sf[:], mt[:], Act.Identity,
                                     bias=sbias, scale=hinvw)
                getattr(nc, S_ENG).tensor_tensor(sb[:], sf[:], qf[:],
                                                 Alu.subtract)

                # d products (w*s1, w*s2, w*s1*s2)
                dspecs = [(cw, 0, s1), (2 * cw, 0, s2), (3 * cw, cw, s2)]
                for di, (o_off, i_off, sig) in enumerate(dspecs):
                    eng = nc.gpsimd if di < D_GP else nc.vector
                    eng.tensor_tensor(dsd[:, o_off:o_off + cw],
                                      dsd[:, i_off:i_off + cw],
                                      sig[:], Alu.mult)

                # parity-gated channels: dsd[4:8] = s * dsd[0:4]
                d_b = dsd[:, 0:4 * cw].rearrange("p (r t) -> p r t", r=NCH)
                s_b = sb[:].rearrange("p (o t) -> p o t", o=1)
                s_b = s_b.to_broadcast((P, NCH, cw))
                sd1 = dsd[:, 4 * cw:8 * cw].rearrange("p (r t) -> p r t",
                                                      r=NCH)
                getattr(nc, SD1_ENG).tensor_tensor(sd1, s_b, d_b, Alu.mult)

                # one-hot slabs over bin pairs + constant ones column
                qoh = qoh_pool.tile([P, QP * cw], dt.bfloat16, tag="qoh")
                for m in range(QP - 1):
                    eng = nc.gpsimd if m < QOH_GP else nc.vector
                    eng.tensor_scalar(qoh[:, m * cw:(m + 1) * cw], qf[:],
                                      128.0 + m, None, Alu.is_equal)
                if ONES_MODE == "act":
                    nc.scalar.activation(qoh[:, (QP - 1) * cw:QP * cw], qf[:],
                                         Act.Identity, bias=1.0, scale=0.0)
                else:
                    nc.vector.memset(qoh[:, (QP - 1) * cw:QP * cw], 1.0)

                qoh_r = qoh[:].rearrange("p (m t) -> p t m", t=cw)
                dsd_r = dsd[:].rearrange("p (j t) -> p t j", t=cw)
                last_chunk = ci == len(chunks) - 1
                for t in range(cw):
                    nc.tensor.matmul(ps[:], qoh_r[:, t, :], dsd_r[:, t, :],
                                     start=(ci == 0 and t == 0),
                                     stop=(last_chunk and t == cw - 1),
                                     skip_group_check=True)

        out_sb = out_pool.tile([QP, 4 * NCH], dt.float32)
        nc.vector.tensor_copy(out_sb[:, 0:2 * NCH], psums["bkg"][:])
        nc.vector.tensor_copy(out_sb[:, 2 * NCH:4 * NCH], psums["sig"][:])
        nc.sync.dma_start(dout[:], out_sb[:])

    nc.compile()
    return nc


def _shard(arr: np.ndarray, core: int, dtype) -> np.ndarray:
    sl = arr[core * NPC:(core + 1) * NPC]
    out = np.zeros(P * COLS, dtype=np.float32)
    out[:NPC] = sl
    return out.reshape(P, COLS).astype(dtype)


def _unmix(h: np.ndarray) -> np.ndarray:
    """h: [25, 8] psum block -> [NBIN, 4] channel hist (H,H1,H2,H12)."""
    M0 = h[:, 0:NCH].copy()          # pair sums (rows 0..23), ones row at 24
    M1 = h[:, NCH:2 * NCH].copy()    # odd-parity partial sums
    M0[QP - 1] = h[QP - 1, 0:NCH] - M0[:QP - 1].sum(axis=0)
    M1[QP - 1] = h[QP - 1, NCH:2 * NCH] - M1[:QP - 1].sum(axis=0)
    out = np.empty((NBIN, NCH), dtype=np.float64)
    out[0::2] = M0 - M1              # even bins
    out[1::2] = M1                   # odd bins
    return out


def _regions(full: np.ndarray) -> np.ndarray:
    """[NBIN, 4] channels (H, H1, H2, H12) -> regions (A,B,C,D) * INT_LUMI."""
    H, H1, H2, H12 = full[:, 0], full[:, 1], full[:, 2], full[:, 3]
    A = H1 - H12
    B = H12
    C = H - H1 - H2 + H12
    D = H2 - H12
    return np.stack([A, B, C, D], axis=-1) * INT_LUMI


def _likelihood(hb: np.ndarray, hs: np.ndarray) -> float:
    """hb/hs: [NBIN, 4] region histograms (A,B,C,D) in float64."""
    from scipy.special import gammaln

    obs_A, obs_B, obs_C, obs_D = hb[:, 0], hb[:, 1], hb[:, 2], hb[:, 3]
    S_A, S_B, S_C, S_D = hs[:, 0], hs[:, 1], hs[:, 2], hs[:, 3]
    mu = 1.0
    # theta = 0, nA/nC/nD = obs_A/obs_C/obs_D
    exp_A = obs_A + mu * S_A
    exp_C = obs_C + mu * S_C
    exp_D = obs_D + mu * S_D
    bkg_SR = obs_A * obs_D / (obs_C + EPS)
    exp_B = bkg_SR + mu * S_B

    def pois(o, e):
        return o * np.log(e + EPS) - e - gammaln(o + 1.0)

    llh = (pois(obs_A, exp_A) + pois(obs_B, exp_B)
           + pois(obs_C, exp_C) + pois(obs_D, exp_D))
    return -float(llh.sum())


_NC_CACHE = None
LAST_RESULTS = None


def kernel(f1_bkg, f2_bkg, mt_bkg, w_bkg, f1_sig, f2_sig, mt_sig, w_sig,
           cut1, cut2, mt_bin_edges):
    global _NC_CACHE, LAST_RESULTS
    import ml_dtypes
    from concourse.bass_utils import run_bass_kernel_spmd

    if _NC_CACHE is None:
        _NC_CACHE = _build_program()
    nc = _NC_CACHE

    bf16 = ml_dtypes.bfloat16
    edges = np.asarray(mt_bin_edges, dtype=np.float64)
    width = float(edges[1] - edges[0])
    e0 = float(edges[0])
    hw = 0.5 / width
    par = np.zeros((P, 8), dtype=np.float32)
    par[:, 0] = -STEEPNESS * float(cut1)
    par[:, 1] = -STEEPNESS * float(cut2)
    par[:, 2] = hw
    par[:, 3] = 127.5 - e0 * hw
    par[:, 4] = 128.0 - e0 * hw

    arrs = {
        "f1_bkg": (f1_bkg, bf16), "f2_bkg": (f2_bkg, bf16),
        "mt_bkg": (mt_bkg, np.float32), "w_bkg": (w_bkg, bf16),
        "f1_sig": (f1_sig, bf16), "f2_sig": (f2_sig, bf16),
        "mt_sig": (mt_sig, np.float32), "w_sig": (w_sig, bf16),
    }
    arrs = {k: (np.asarray(v, dtype=np.float32), t) for k, (v, t) in
            arrs.items()}

    in_maps = []
    for core in range(NCORES):
        m = {k: _shard(v, core, t) for k, (v, t) in arrs.items()}
        m["params"] = par
        in_maps.append(m)

    try:
        res = run_bass_kernel_spmd(nc, in_maps, core_ids=list(range(NCORES)))
    except Exception:
        # transient device states typically clear on retry
        res = run_bass_kernel_spmd(nc, in_maps, core_ids=list(range(NCORES)))
    LAST_RESULTS = res

    total = np.zeros((QP, 4 * NCH), dtype=np.float64)
    for rmap in res.results:
        total += rmap["hist_out"].astype(np.float64)

    hb = _regions(_unmix(total[:, 0:2 * NCH]))
    hs = _regions(_unmix(total[:, 2 * NCH:4 * NCH]))
    out = _likelihood(hb, hs)
    return np.float32(out)


# revision 7
# speedup vs baseline: 1.4030x; 1.2438x over previous
"""Trainium2 Bass kernel for nn_CLsLoss (ABCD soft-region weighted histograms +
profile likelihood).

Strategy (data-parallel over events, 8 cores):
  - Each core gets 1/8 of the 4M bkg and 1/8 of the 4M sig events as
    [128, 3908] tiles (tail zero-weighted). f1/f2/w are fed as bf16 from the
    host; mt stays fp32 (bin-edge precision).
  - Radix-2 histogram: q = idx>>1 in bf16 via the +256 rint trick
    (qf = rint(mt*invw/2 + 127.5), sf = rint(mt*invw/2 + 128) so
    s = sf - qf = idx&1, all exact in bf16's [128,256) unit-ULP window).
  - Stationary per event-column: 24 is_equal(qf, 128+m) one-hot slabs plus a
    constant ones column (bin-pair 24 is recovered on host from the ones row).
  - Moving operand: 8 channels [w, w*s1, w*s2, w*s1*s2] and the same four
    gated by s (odd-parity partial sums). Even-parity bins are recovered on
    host as M0 - M1 (no even-gated products needed on device).
  - TensorE: psum[25, 8] += qoh[128,25]^T @ dsd[128,8] per column, one PSUM
    fp32 accumulation group per dataset.
  - Host: sum per-core [25,16] partials in float64, unmix parity + ones row,
    derive regions A/B/C/D, evaluate the [50]-bin profile likelihood.
"""

import os as _os

import numpy as np

NBIN = 50
N_EVENTS = 4_000_000
NCORES = 8
NPC = N_EVENTS // NCORES          # 500_000 events per core per dataset
P = 128
COLS = 3908                       # 128*3908 = 500224 >= NPC (tail zero-weighted)
QP = 25                           # bin pairs (q = idx >> 1)
NCH = 4                           # weight channels: 1, s1, s2, s1*s2
INT_LUMI = 117100.0
EPS = 1e-6
STEEPNESS = 20.0

CHUNK = int(_os.environ.get("K_CHUNK", "1303"))
QOH_GP = _os.environ.get("K_QOH_GP", "2,3")          # qoh slabs on GpSimd (cycle)
RAMP = _os.environ.get("K_RAMP", "326,978")          # leading warmup chunks
S_ENG = _os.environ.get("K_S_ENG", "vector")
ONES_MODE = _os.environ.get("K_ONES", "act")
MMSTEP = int(_os.environ.get("K_MMSTEP", "1"))       # diagnostic only


def _build_program():
    import concourse.bass as bass
    import concourse.bacc as bacc
    import concourse.mybir as mybir
    import concourse.tile as tile

    dt = mybir.dt
    Alu = mybir.AluOpType
    Act = mybir.ActivationFunctionType

    nc = bacc.Bacc("TRN2", target_bir_lowering=False, debug=False,
                   num_devices=NCORES)

    din = {}
    for ds in ("bkg", "sig"):
        din[f"f1_{ds}"] = nc.dram_tensor(f"f1_{ds}", [P, COLS], dt.bfloat16,
                                         kind="ExternalInput")
        din[f"f2_{ds}"] = nc.dram_tensor(f"f2_{ds}", [P, COLS], dt.bfloat16,
                                         kind="ExternalInput")
        din[f"mt_{ds}"] = nc.dram_tensor(f"mt_{ds}", [P, COLS], dt.float32,
                                         kind="ExternalInput")
        din[f"w_{ds}"] = nc.dram_tensor(f"w_{ds}", [P, COLS], dt.bfloat16,
                                        kind="ExternalInput")
    dpar = nc.dram_tensor("params", [P, 8], dt.float32, kind="ExternalInput")
    dout = nc.dram_tensor("hist_out", [QP, 4 * NCH], dt.float32,
                          kind="ExternalOutput")

    def make_chunks(ramp):
        out, c0 = [], 0
        for r in ramp:
            if c0 + r >= COLS:
                break
            out.append((c0, r))
            c0 += r
        while c0 < COLS:
            out.append((c0, min(CHUNK, COLS - c0)))
            c0 += CHUNK
        return out

    ramp = [int(x) for x in RAMP.split(",") if x]
    chunk_sets = {"bkg": make_chunks(ramp), "sig": make_chunks([])}
    qoh_gp_cycle = [int(x) for x in QOH_GP.split(",") if x]

    from contextlib import ExitStack
    with tile.TileContext(nc) as tc, ExitStack() as ctx:
        io_pool = ctx.enter_context(tc.tile_pool(name="io", bufs=2))
        mid_pool = ctx.enter_context(tc.tile_pool(name="mid", bufs=2))
        qoh_pool = ctx.enter_context(tc.tile_pool(name="qoh", bufs=2))
        const_pool = ctx.enter_context(tc.tile_pool(name="const", bufs=1))
        psum_pool = ctx.enter_context(
            tc.tile_pool(name="psum", bufs=1, space=bass.MemorySpace.PSUM))
        out_pool = ctx.enter_context(tc.tile_pool(name="out", bufs=1))

        par = const_pool.tile([P, 8], dt.float32)
        nc.sync.dma_start(par[:], dpar[:])
        bias1 = par[:, 0:1]      # -20*cut1
        bias2 = par[:, 1:2]      # -20*cut2
        hinvw = par[:, 2:3]      # 0.5/bin_width
        qbias = par[:, 3:4]      # 127.5 - e0/(2w)
        sbias = par[:, 4:5]      # 128.0 - e0/(2w)

        psums = {ds: psum_pool.tile([QP, 2 * NCH], dt.float32,
                                    name=f"ps_{ds}", tag=f"ps_{ds}")
                 for ds in ("bkg", "sig")}

        for ds in ("bkg", "sig"):
            ps = psums[ds]
            chunks = chunk_sets[ds]
            gi = 0
            for ci, (c0, cw) in enumerate(chunks):
                f1 = io_pool.tile([P, cw], dt.bfloat16, tag="f1", bufs=1)
                f2 = io_pool.tile([P, cw], dt.bfloat16, tag="f2")
                mt = io_pool.tile([P, cw], dt.float32, tag="mt")
                # dsd channels: w, w*s1, w*s2, w*s1*s2, then the same with
                # w replaced by ws = w*s (parity-gated chain)
                dsd = mid_pool.tile([P, 8 * cw], dt.bfloat16, tag="dsd")
                nc.sync.dma_start(f1[:], din[f"f1_{ds}"][:, c0:c0 + cw])
                nc.sync.dma_start(f2[:], din[f"f2_{ds}"][:, c0:c0 + cw])
                nc.sync.dma_start(mt[:], din[f"mt_{ds}"][:, c0:c0 + cw])
                nc.sync.dma_start(dsd[:, 0:cw], din[f"w_{ds}"][:, c0:c0 + cw])

                s12 = mid_pool.tile([P, 2 * cw], dt.bfloat16, tag="s12")
                qf = mid_pool.tile([P, cw], dt.bfloat16, tag="qf")
                sf = mid_pool.tile([P, cw], dt.bfloat16, tag="sf", bufs=1)
                sb = mid_pool.tile([P, cw], dt.bfloat16, tag="sb", bufs=1)
                nc.scalar.activation(qf[:], mt[:], Act.Identity,
                                     bias=qbias, scale=hinvw)
                nc.scalar.activation(sf[:], mt[:], Act.Identity,
                                     bias=sbias, scale=hinvw)
                nc.scalar.activation(s12[:, 0:cw], f1[:], Act.Sigmoid,
                                     bias=bias1, scale=STEEPNESS)
                nc.scalar.activation(s12[:, cw:2 * cw], f2[:], Act.Sigmoid,
                                     bias=bias2, scale=STEEPNESS)
                nc.vector.tensor_tensor(sb[:], sf[:], qf[:], Alu.subtract)

                # ungated chain on GpSimd: [d1|d2] = w*(s1|s2), d12 = d1*s2
                w_b = dsd[:, 0:cw].rearrange("p (o t) -> p o t", o=1)
                w_b = w_b.to_broadcast((P, 2, cw))
                s12_r = s12[:].rearrange("p (r t) -> p r t", r=2)
                nc.gpsimd.tensor_tensor(
                    dsd[:, cw:3 * cw].rearrange("p (r t) -> p r t", r=2),
                    w_b, s12_r, Alu.mult)
                nc.gpsimd.tensor_tensor(dsd[:, 3 * cw:4 * cw],
                                        dsd[:, cw:2 * cw],
                                        s12[:, cw:2 * cw], Alu.mult)

                # gated chain on DVE: ws = w*s, [g1|g2] = ws*(s1|s2),
                # g12 = g1*s2
                nc.vector.tensor_tensor(dsd[:, 4 * cw:5 * cw],
                                        dsd[:, 0:cw], sb[:], Alu.mult)
                ws_b = dsd[:, 4 * cw:5 * cw].rearrange("p (o t) -> p o t",
                                                       o=1)
                ws_b = ws_b.to_broadcast((P, 2, cw))
                nc.vector.tensor_tensor(
                    dsd[:, 5 * cw:7 * cw].rearrange("p (r t) -> p r t", r=2),
                    ws_b, s12_r, Alu.mult)
                nc.vector.tensor_tensor(dsd[:, 7 * cw:8 * cw],
                                        dsd[:, 5 * cw:6 * cw],
                                        s12[:, cw:2 * cw], Alu.mult)

                # one-hot slabs over bin pairs + constant ones column
                qoh = qoh_pool.tile([P, QP * cw], dt.bfloat16, tag="qoh")
                n_gp = qoh_gp_cycle[gi % len(qoh_gp_cycle)]
                gi += 1
                for m in range(QP - 1):
                    eng = nc.gpsimd if m < n_gp else nc.vector
                    eng.tensor_scalar(qoh[:, m * cw:(m + 1) * cw], qf[:],
                                      128.0 + m, None, Alu.is_equal)
                if ONES_MODE == "act":
                    nc.scalar.activation(qoh[:, (QP - 1) * cw:QP * cw], qf[:],
                                         Act.Identity, bias=1.0, scale=0.0)
                else:
                    nc.vector.memset(qoh[:, (QP - 1) * cw:QP * cw], 1.0)

                qoh_r = qoh[:].rearrange("p (m t) -> p t m", t=cw)
                dsd_r = dsd[:].rearrange("p (j t) -> p t j", t=cw)
                last_chunk = ci == len(chunks) - 1
                for t in range(0, cw, MMSTEP):
                    nc.tensor.matmul(ps[:], qoh_r[:, t, :], dsd_r[:, t, :],
                                     start=(ci == 0 and t == 0),
                                     stop=(last_chunk and t >= cw - MMSTEP),
                                     skip_group_check=True)

        out_sb = out_pool.tile([QP, 4 * NCH], dt.float32)
        nc.vector.tensor_copy(out_sb[:, 0:2 * NCH], psums["bkg"][:])
        nc.vector.tensor_copy(out_sb[:, 2 * NCH:4 * NCH], psums["sig"][:])
        nc.sync.dma_start(dout[:], out_sb[:])

    nc.compile()
    return nc


def _shard(arr: np.ndarray, core: int, dtype) -> np.ndarray:
    sl = arr[core * NPC:(core + 1) * NPC]
    out = np.zeros(P * COLS, dtype=np.float32)
    out[:NPC] = sl
    return out.reshape(P, COLS).astype(dtype)


def _unmix(h: np.ndarray) -> np.ndarray:
    """h: [25, 8] psum block -> [NBIN, 4] channel hist (H,H1,H2,H12)."""
    M0 = h[:, 0:NCH].copy()          # pair sums (rows 0..23), ones row at 24
    M1 = h[:, NCH:2 * NCH].copy()    # odd-parity partial sums
    M0[QP - 1] = h[QP - 1, 0:NCH] - M0[:QP - 1].sum(axis=0)
    M1[QP - 1] = h[QP - 1, NCH:2 * NCH] - M1[:QP - 1].sum(axis=0)
    out = np.empty((NBIN, NCH), dtype=np.float64)
    out[0::2] = M0 - M1              # even bins
    out[1::2] = M1                   # odd bins
    return out


def _regions(full: np.ndarray) -> np.ndarray:
    """[NBIN, 4] channels (H, H1, H2, H12) -> regions (A,B,C,D) * INT_LUMI."""
    H, H1, H2, H12 = full[:, 0], full[:, 1], full[:, 2], full[:, 3]
    A = H1 - H12
    B = H12
    C = H - H1 - H2 + H12
    D = H2 - H12
    return np.stack([A, B, C, D], axis=-1) * INT_LUMI


def _likelihood(hb: np.ndarray, hs: np.ndarray) -> float:
    """hb/hs: [NBIN, 4] region histograms (A,B,C,D) in float64."""
    from scipy.special import gammaln

    obs_A, obs_B, obs_C, obs_D = hb[:, 0], hb[:, 1], hb[:, 2], hb[:, 3]
    S_A, S_B, S_C, S_D = hs[:, 0], hs[:, 1], hs[:, 2], hs[:, 3]
    mu = 1.0
    # theta = 0, nA/nC/nD = obs_A/obs_C/obs_D
    exp_A = obs_A + mu * S_A
    exp_C = obs_C + mu * S_C
    exp_D = obs_D + mu * S_D
    bkg_SR = obs_A * obs_D / (obs_C + EPS)
    exp_B = bkg_SR + mu * S_B

    def pois(o, e):
        return o * np.log(e + EPS) - e - gammaln(o + 1.0)

    llh = (pois(obs_A, exp_A) + pois(obs_B, exp_B)
           + pois(obs_C, exp_C) + pois(obs_D, exp_D))
    return -float(llh.sum())


_NC_CACHE = None
LAST_RESULTS = None


def kernel(f1_bkg, f2_bkg, mt_bkg, w_bkg, f1_sig, f2_sig, mt_sig, w_sig,
           cut1, cut2, mt_bin_edges):
    global _NC_CACHE, LAST_RESULTS
    import ml_dtypes
    from concourse.bass_utils import run_bass_kernel_spmd

    if _NC_CACHE is None:
        _NC_CACHE = _build_program()
    nc = _NC_CACHE

    bf16 = ml_dtypes.bfloat16
    edges = np.asarray(mt_bin_edges, dtype=np.float64)
    width = float(edges[1] - edges[0])
    e0 = float(edges[0])
    hw = 0.5 / width
    par = np.zeros((P, 8), dtype=np.float32)
    par[:, 0] = -STEEPNESS * float(cut1)
    par[:, 1] = -STEEPNESS * float(cut2)
    par[:, 2] = hw
    par[:, 3] = 127.5 - e0 * hw
    par[:, 4] = 128.0 - e0 * hw

    arrs = {
        "f1_bkg": (f1_bkg, bf16), "f2_bkg": (f2_bkg, bf16),
        "mt_bkg": (mt_bkg, np.float32), "w_bkg": (w_bkg, bf16),
        "f1_sig": (f1_sig, bf16), "f2_sig": (f2_sig, bf16),
        "mt_sig": (mt_sig, np.float32), "w_sig": (w_sig, bf16),
    }
    arrs = {k: (np.asarray(v, dtype=np.float32), t) for k, (v, t) in
            arrs.items()}

    in_maps = []
    for core in range(NCORES):
        m = {k: _shard(v, core, t) for k, (v, t) in arrs.items()}
        m["params"] = par
        in_maps.append(m)

    try:
        res = run_bass_kernel_spmd(nc, in_maps, core_ids=list(range(NCORES)))
    except Exception:
        # transient device states typically clear on retry
        res = run_bass_kernel_spmd(nc, in_maps, core_ids=list(range(NCORES)))
    LAST_RESULTS = res

    total = np.zeros((QP, 4 * NCH), dtype=np.float64)
    for rmap in res.results:
        total += rmap["hist_out"].astype(np.float64)

    hb = _regions(_unmix(total[:, 0:2 * NCH]))
    hs = _regions(_unmix(total[:, 2 * NCH:4 * NCH]))
    out = _likelihood(hb, hs)
    return np.float32(out)


# revision 8
# speedup vs baseline: 1.4527x; 1.0354x over previous
"""Trainium2 Bass kernel for nn_CLsLoss (ABCD soft-region weighted histograms +
profile likelihood).

Strategy (data-parallel over events, 8 cores):
  - Each core gets 1/8 of the 4M bkg and 1/8 of the 4M sig events as
    [128, 3908] tiles (tail zero-weighted). f1/f2/w are fed as bf16 from the
    host; mt stays fp32 (bin-edge precision).
  - Radix-2 histogram: q = idx>>1 in bf16 via the +256 rint trick
    (qf = rint(mt*invw/2 + 127.5), sf = rint(mt*invw/2 + 128) so
    s = sf - qf = idx&1, all exact in bf16's [128,256) unit-ULP window).
  - Stationary per event-column: 24 is_equal(qf, 128+m) one-hot slabs plus a
    constant ones column (bin-pair 24 is recovered on host from the ones row).
  - Moving operand: 8 channels [w, w*s1, w*s2, w*s1*s2] and the same four
    gated by s (odd-parity partial sums). Even-parity bins are recovered on
    host as M0 - M1 (no even-gated products needed on device).
  - TensorE: psum[25, 8] += qoh[128,25]^T @ dsd[128,8] per column, one PSUM
    fp32 accumulation group per dataset.
  - Host: sum per-core [25,16] partials in float64, unmix parity + ones row,
    derive regions A/B/C/D, evaluate the [50]-bin profile likelihood.
"""

import os as _os

import numpy as np

NBIN = 50
N_EVENTS = 4_000_000
NCORES = 8
NPC = N_EVENTS // NCORES          # 500_000 events per core per dataset
P = 128
COLS = 3908                       # 128*3908 = 500224 >= NPC (tail zero-weighted)
QP = 25                           # bin pairs (q = idx >> 1)
NCH = 4                           # weight channels: 1, s1, s2, s1*s2
INT_LUMI = 117100.0
EPS = 1e-6
STEEPNESS = 20.0

CHUNK = int(_os.environ.get("K_CHUNK", "1303"))
QOH_GP = _os.environ.get("K_QOH_GP", "2,3")          # qoh slabs on GpSimd (cycle)
RAMP = _os.environ.get("K_RAMP", "326,978")          # leading warmup chunks
RAMPOUT = _os.environ.get("K_RAMPOUT", "978,326")    # trailing drain chunks
S_ENG = _os.environ.get("K_S_ENG", "vector")
ONES_MODE = _os.environ.get("K_ONES", "act")
MMSTEP = int(_os.environ.get("K_MMSTEP", "1"))       # diagnostic only


def _build_program():
    import concourse.bass as bass
    import concourse.bacc as bacc
    import concourse.mybir as mybir
    import concourse.tile as tile

    dt = mybir.dt
    Alu = mybir.AluOpType
    Act = mybir.ActivationFunctionType

    nc = bacc.Bacc("TRN2", target_bir_lowering=False, debug=False,
                   num_devices=NCORES)

    din = {}
    for ds in ("bkg", "sig"):
        din[f"f1_{ds}"] = nc.dram_tensor(f"f1_{ds}", [P, COLS], dt.bfloat16,
                                         kind="ExternalInput")
        din[f"f2_{ds}"] = nc.dram_tensor(f"f2_{ds}", [P, COLS], dt.bfloat16,
                                         kind="ExternalInput")
        din[f"mt_{ds}"] = nc.dram_tensor(f"mt_{ds}", [P, COLS], dt.float32,
                                         kind="ExternalInput")
        din[f"w_{ds}"] = nc.dram_tensor(f"w_{ds}", [P, COLS], dt.bfloat16,
                                        kind="ExternalInput")
    dpar = nc.dram_tensor("params", [P, 8], dt.float32, kind="ExternalInput")
    dout = nc.dram_tensor("hist_out", [QP, 4 * NCH], dt.float32,
                          kind="ExternalOutput")

    def make_chunks(ramp, rampout):
        head, c0 = [], 0
        for r in ramp:
            head.append((c0, r))
            c0 += r
        tail_widths = list(rampout)
        c1 = COLS - sum(tail_widths)
        mid, cm = [], c0
        while cm < c1:
            mid.append((cm, min(CHUNK, c1 - cm)))
            cm += CHUNK
        tail = []
        for r in tail_widths:
            tail.append((c1, r))
            c1 += r
        return head + mid + tail

    ramp = [int(x) for x in RAMP.split(",") if x]
    rampout = [int(x) for x in RAMPOUT.split(",") if x]
    chunk_sets = {"bkg": make_chunks(ramp, []),
                  "sig": make_chunks([], rampout)}
    qoh_gp_cycle = [int(x) for x in QOH_GP.split(",") if x]

    from contextlib import ExitStack
    with tile.TileContext(nc) as tc, ExitStack() as ctx:
        io_pool = ctx.enter_context(tc.tile_pool(name="io", bufs=2))
        mid_pool = ctx.enter_context(tc.tile_pool(name="mid", bufs=2))
        qoh_pool = ctx.enter_context(tc.tile_pool(name="qoh", bufs=2))
        const_pool = ctx.enter_context(tc.tile_pool(name="const", bufs=1))
        psum_pool = ctx.enter_context(
            tc.tile_pool(name="psum", bufs=1, space=bass.MemorySpace.PSUM))
        out_pool = ctx.enter_context(tc.tile_pool(name="out", bufs=1))

        par = const_pool.tile([P, 8], dt.float32)
        nc.sync.dma_start(par[:], dpar[:])
        bias1 = par[:, 0:1]      # -20*cut1
        bias2 = par[:, 1:2]      # -20*cut2
        hinvw = par[:, 2:3]      # 0.5/bin_width
        qbias = par[:, 3:4]      # 127.5 - e0/(2w)
        sbias = par[:, 4:5]      # 128.0 - e0/(2w)

        psums = {ds: psum_pool.tile([QP, 2 * NCH], dt.float32,
                                    name=f"ps_{ds}", tag=f"ps_{ds}")
                 for ds in ("bkg", "sig")}

        for ds in ("bkg", "sig"):
            ps = psums[ds]
            chunks = chunk_sets[ds]
            gi = 0
            for ci, (c0, cw) in enumerate(chunks):
                f1 = io_pool.tile([P, cw], dt.bfloat16, tag="f1", bufs=1)
                f2 = io_pool.tile([P, cw], dt.bfloat16, tag="f2")
                mt = io_pool.tile([P, cw], dt.float32, tag="mt")
                # dsd channels: w, w*s1, w*s2, w*s1*s2, then the same with
                # w replaced by ws = w*s (parity-gated chain)
                dsd = mid_pool.tile([P, 8 * cw], dt.bfloat16, tag="dsd")
                nc.sync.dma_start(f1[:], din[f"f1_{ds}"][:, c0:c0 + cw])
                nc.sync.dma_start(f2[:], din[f"f2_{ds}"][:, c0:c0 + cw])
                nc.sync.dma_start(mt[:], din[f"mt_{ds}"][:, c0:c0 + cw])
                nc.sync.dma_start(dsd[:, 0:cw], din[f"w_{ds}"][:, c0:c0 + cw])

                s12 = mid_pool.tile([P, 2 * cw], dt.bfloat16, tag="s12")
                qf = mid_pool.tile([P, cw], dt.bfloat16, tag="qf")
                sf = mid_pool.tile([P, cw], dt.bfloat16, tag="sf", bufs=1)
                sb = mid_pool.tile([P, cw], dt.bfloat16, tag="sb", bufs=1)
                nc.scalar.activation(qf[:], mt[:], Act.Identity,
                                     bias=qbias, scale=hinvw)
                nc.scalar.activation(sf[:], mt[:], Act.Identity,
                                     bias=sbias, scale=hinvw)
                nc.scalar.activation(s12[:, 0:cw], f1[:], Act.Sigmoid,
                                     bias=bias1, scale=STEEPNESS)
                nc.scalar.activation(s12[:, cw:2 * cw], f2[:], Act.Sigmoid,
                                     bias=bias2, scale=STEEPNESS)
                nc.vector.tensor_tensor(sb[:], sf[:], qf[:], Alu.subtract)

                # ungated chain on GpSimd: [d1|d2] = w*(s1|s2), d12 = d1*s2
                w_b = dsd[:, 0:cw].rearrange("p (o t) -> p o t", o=1)
                w_b = w_b.to_broadcast((P, 2, cw))
                s12_r = s12[:].rearrange("p (r t) -> p r t", r=2)
                nc.gpsimd.tensor_tensor(
                    dsd[:, cw:3 * cw].rearrange("p (r t) -> p r t", r=2),
                    w_b, s12_r, Alu.mult)
                nc.gpsimd.tensor_tensor(dsd[:, 3 * cw:4 * cw],
                                        dsd[:, cw:2 * cw],
                                        s12[:, cw:2 * cw], Alu.mult)

                # gated chain on DVE: ws = w*s, [g1|g2] = ws*(s1|s2),
                # g12 = g1*s2
                nc.vector.tensor_tensor(dsd[:, 4 * cw:5 * cw],
                                        dsd[:, 0:cw], sb[:], Alu.mult)
                ws_b = dsd[:, 4 * cw:5 * cw].rearrange("p (o t) -> p o t",
                                                       o=1)
                ws_b = ws_b.to_broadcast((P, 2, cw))
                nc.vector.tensor_tensor(
                    dsd[:, 5 * cw:7 * cw].rearrange("p (r t) -> p r t", r=2),
                    ws_b, s12_r, Alu.mult)
                nc.vector.tensor_tensor(dsd[:, 7 * cw:8 * cw],
                                        dsd[:, 5 * cw:6 * cw],
                                        s12[:, cw:2 * cw], Alu.mult)

                # one-hot slabs over bin pairs + constant ones column
                qoh = qoh_pool.tile([P, QP * cw], dt.bfloat16, tag="qoh")
                n_gp = qoh_gp_cycle[gi % len(qoh_gp_cycle)]
                gi += 1
                for m in range(QP - 1):
                    eng = nc.gpsimd if m < n_gp else nc.vector
                    eng.tensor_scalar(qoh[:, m * cw:(m + 1) * cw], qf[:],
                                      128.0 + m, None, Alu.is_equal)
                if ONES_MODE == "act":
                    nc.scalar.activation(qoh[:, (QP - 1) * cw:QP * cw], qf[:],
                                         Act.Identity, bias=1.0, scale=0.0)
                else:
                    nc.vector.memset(qoh[:, (QP - 1) * cw:QP * cw], 1.0)

                qoh_r = qoh[:].rearrange("p (m t) -> p t m", t=cw)
                dsd_r = dsd[:].rearrange("p (j t) -> p t j", t=cw)
                last_chunk = ci == len(chunks) - 1
                for t in range(0, cw, MMSTEP):
                    nc.tensor.matmul(ps[:], qoh_r[:, t, :], dsd_r[:, t, :],
                                     start=(ci == 0 and t == 0),
                                     stop=(last_chunk and t >= cw - MMSTEP),
                                     skip_group_check=True)

        out_sb = out_pool.tile([QP, 4 * NCH], dt.float32)
        nc.vector.tensor_copy(out_sb[:, 0:2 * NCH], psums["bkg"][:])
        nc.vector.tensor_copy(out_sb[:, 2 * NCH:4 * NCH], psums["sig"][:])
        nc.sync.dma_start(dout[:], out_sb[:])

    nc.compile()
    return nc


def _shard(arr: np.ndarray, core: int, dtype) -> np.ndarray:
    sl = arr[core * NPC:(core + 1) * NPC]
    out = np.zeros(P * COLS, dtype=np.float32)
    out[:NPC] = sl
    return out.reshape(P, COLS).astype(dtype)


def _unmix(h: np.ndarray) -> np.ndarray:
    """h: [25, 8] psum block -> [NBIN, 4] channel hist (H,H1,H2,H12)."""
    M0 = h[:, 0:NCH].copy()          # pair sums (rows 0..23), ones row at 24
    M1 = h[:, NCH:2 * NCH].copy()    # odd-parity partial sums
    M0[QP - 1] = h[QP - 1, 0:NCH] - M0[:QP - 1].sum(axis=0)
    M1[QP - 1] = h[QP - 1, NCH:2 * NCH] - M1[:QP - 1].sum(axis=0)
    out = np.empty((NBIN, NCH), dtype=np.float64)
    out[0::2] = M0 - M1              # even bins
    out[1::2] = M1                   # odd bins
    return out


def _regions(full: np.ndarray) -> np.ndarray:
    """[NBIN, 4] channels (H, H1, H2, H12) -> regions (A,B,C,D) * INT_LUMI."""
    H, H1, H2, H12 = full[:, 0], full[:, 1], full[:, 2], full[:, 3]
    A = H1 - H12
    B = H12
    C = H - H1 - H2 + H12
    D = H2 - H12
    return np.stack([A, B, C, D], axis=-1) * INT_LUMI


def _likelihood(hb: np.ndarray, hs: np.ndarray) -> float:
    """hb/hs: [NBIN, 4] region histograms (A,B,C,D) in float64."""
    from scipy.special import gammaln

    obs_A, obs_B, obs_C, obs_D = hb[:, 0], hb[:, 1], hb[:, 2], hb[:, 3]
    S_A, S_B, S_C, S_D = hs[:, 0], hs[:, 1], hs[:, 2], hs[:, 3]
    mu = 1.0
    # theta = 0, nA/nC/nD = obs_A/obs_C/obs_D
    exp_A = obs_A + mu * S_A
    exp_C = obs_C + mu * S_C
    exp_D = obs_D + mu * S_D
    bkg_SR = obs_A * obs_D / (obs_C + EPS)
    exp_B = bkg_SR + mu * S_B

    def pois(o, e):
        return o * np.log(e + EPS) - e - gammaln(o + 1.0)

    llh = (pois(obs_A, exp_A) + pois(obs_B, exp_B)
           + pois(obs_C, exp_C) + pois(obs_D, exp_D))
    return -float(llh.sum())


_NC_CACHE = None
LAST_RESULTS = None


def kernel(f1_bkg, f2_bkg, mt_bkg, w_bkg, f1_sig, f2_sig, mt_sig, w_sig,
           cut1, cut2, mt_bin_edges):
    global _NC_CACHE, LAST_RESULTS
    import ml_dtypes
    from concourse.bass_utils import run_bass_kernel_spmd

    if _NC_CACHE is None:
        _NC_CACHE = _build_program()
    nc = _NC_CACHE

    bf16 = ml_dtypes.bfloat16
    edges = np.asarray(mt_bin_edges, dtype=np.float64)
    width = float(edges[1] - edges[0])
    e0 = float(edges[0])
    hw = 0.5 / width
    par = np.zeros((P, 8), dtype=np.float32)
    par[:, 0] = -STEEPNESS * float(cut1)
    par[:, 1] = -STEEPNESS * float(cut2)
    par[:, 2] = hw
    par[:, 3] = 127.5 - e0 * hw
    par[:, 4] = 128.0 - e0 * hw

    arrs = {
        "f1_bkg": (f1_bkg, bf16), "f2_bkg": (f2_bkg, bf16),
        "mt_bkg": (mt_bkg, np.float32), "w_bkg": (w_bkg, bf16),
        "f1_sig": (f1_sig, bf16), "f2_sig": (f2_sig, bf16),
        "mt_sig": (mt_sig, np.float32), "w_sig": (w_sig, bf16),
    }
    arrs = {k: (np.asarray(v, dtype=np.float32), t) for k, (v, t) in
            arrs.items()}

    in_maps = []
    for core in range(NCORES):
        m = {k: _shard(v, core, t) for k, (v, t) in arrs.items()}
        m["params"] = par
        in_maps.append(m)

    try:
        res = run_bass_kernel_spmd(nc, in_maps, core_ids=list(range(NCORES)))
    except Exception:
        # transient device states typically clear on retry
        res = run_bass_kernel_spmd(nc, in_maps, core_ids=list(range(NCORES)))
    LAST_RESULTS = res

    total = np.zeros((QP, 4 * NCH), dtype=np.float64)
    for rmap in res.results:
        total += rmap["hist_out"].astype(np.float64)

    hb = _regions(_unmix(total[:, 0:2 * NCH]))
    hs = _regions(_unmix(total[:, 2 * NCH:4 * NCH]))
    out = _likelihood(hb, hs)
    return np.float32(out)
